# revision 4
# baseline (speedup 1.0000x reference)
"""DeltaRule memory scan kernel for Trainium2, 8 NeuronCores, data-parallel over batch.

Reference semantics (per batch element, H=512, L=2048):
    M_0 = 0  [H,H]
    for t in 0..L-2:   k = hidden[t]
        d = k.k + eps; delta = k - (M k)/d; M += outer(delta, k)
    out = (M @ hidden[L-1]) @ W.T + b

Implementation: superchunks of 256 keys processed as 2x128 blocks (UT transform
per 128-block + cross-block correction). All state matmuls run in float32r
(1 cycle/row at free-dim >= 256); the intra-block inverse chain runs in bf16.

Per superchunk with K0,K1 [128,H], r_i = 1/(rowsum(Ki^2)+eps):
    G0 = K0 [K0|K1]^T, G1 = K1 [K0|K1]^T          (pair gram, f32r, ap256)
    d_i = diag block of G_i ; A_i = strict_tril(diag(r_i) G_ii) ; PT = G0[:,128:]
    T_i^T ~= Neumann chain of A_i^T (exact through A^31)
    U0 = T0 (K0 - diag(r0) K0 M^T)
    V1 = K1 M^T + PT^T U0 ;  U1 = T1 (K1 - diag(r1) V1)
    M^T += K0^T U0 + K1^T U1                       (PSUM-merged, one add per jb)
"""
import sys
import numpy as np
from contextlib import ExitStack

sys.path.insert(0, "/opt/trn_rl_repo")

import concourse.bass as bass
import concourse.mybir as mybir
from concourse import tile
from concourse.bass_utils import run_bass_kernel_spmd
from concourse.masks import make_identity, make_lower_triangular

B, L, H = 32, 2048, 512
NCORES = 8
BPC = B // NCORES          # 4 batch elements per core
SC = 256                   # superchunk (2 blocks of 128)
T = L - 1                  # 2047 scan steps
NSC = (T + SC - 1) // SC   # 8 (last superchunk has 255 valid rows)
NLEV = 3                   # Neumann levels -> exact through A^15 (err ~||A||^16)
EPS = 1e-6
HB = H // 128              # 4 h-blocks

f32 = mybir.dt.float32
f32r = mybir.dt.float32r
bf16 = mybir.dt.bfloat16
MULT = mybir.AluOpType.mult
ADD = mybir.AluOpType.add

_cached = {}


def _build_program():
    nc = bass.Bass(target_bir_lowering=False, debug=False)

    hidden_d = nc.declare_dram_parameter("hidden", [BPC, L, H], f32r, isOutput=False)
    w_d = nc.declare_dram_parameter("W", [H, H], f32, isOutput=False)
    b_d = nc.declare_dram_parameter("bvec", [H], f32, isOutput=False)
    zrow_d = nc.declare_dram_parameter("zrow", [1, H], f32r, isOutput=False)
    out_d = nc.declare_dram_parameter("out", [BPC, H], f32, isOutput=True)

    with tile.TileContext(nc) as tc, ExitStack() as ctx:
        consts = ctx.enter_context(tc.tile_pool(name="consts", bufs=1))
        wbuild = ctx.enter_context(tc.tile_pool(name="wbuild", bufs=2))
        mtpool = ctx.enter_context(tc.tile_pool(name="mt", bufs=1))
        kpool = ctx.enter_context(tc.tile_pool(name="k", bufs=2))
        ktpool = ctx.enter_context(tc.tile_pool(name="kt", bufs=2))
        chain = ctx.enter_context(tc.tile_pool(name="chain", bufs=3))
        gfpool = ctx.enter_context(tc.tile_pool(name="gf", bufs=2))
        upool = ctx.enter_context(tc.tile_pool(name="u", bufs=1))
        ptpool = ctx.enter_context(tc.tile_pool(name="pt", bufs=2))
        small = ctx.enter_context(tc.tile_pool(name="small", bufs=2))
        psgram = ctx.enter_context(tc.tile_pool(name="psgram", bufs=3, space="PSUM"))
        pshi = ctx.enter_context(tc.tile_pool(name="pshi", bufs=5, space="PSUM"))

        # ---- constants ----
        ident_f = consts.tile([128, 128], f32, tag="identf")
        make_identity(nc, ident_f[:])
        ident_b = consts.tile([128, 128], bf16, tag="identb")
        make_identity(nc, ident_b[:])
        ident_r = consts.tile([128, 128], f32r, tag="identr")
        nc.scalar.copy(ident_r[:], ident_f[:])
        # packed identity: I in each of the 4 b-slices
        identp = consts.tile([128, H], bf16, tag="identp")
        for bi in range(BPC):
            nc.vector.tensor_copy(identp[:, bi * 128:(bi + 1) * 128], ident_b[:])
        smask = consts.tile([128, 128], f32, tag="smask")
        make_lower_triangular(nc, smask[:], val=1.0, diag=False)

        # W^T: WT[ib][i', o] = W[o, ib*128+i']  (f32, used once in the finale)
        wt = [consts.tile([128, H], f32, tag=f"wt{ib}", name=f"wt{ib}") for ib in range(HB)]
        for op in range(HB):
            wsb = wbuild.tile([128, H], f32, tag="wsb")
            nc.sync.dma_start(wsb[:], w_d[op * 128:(op + 1) * 128, :])
            for ib in range(HB):
                tps = psgram.tile([128, 128], f32, tag="gram")
                nc.tensor.transpose(tps[:], wsb[:, ib * 128:(ib + 1) * 128], ident_f[:])
                nc.scalar.copy(wt[ib][:, op * 128:(op + 1) * 128], tps[:])

        bias_row = consts.tile([1, H], f32, tag="biasrow")
        nc.sync.dma_start(bias_row[:], b_d[None, :])

        # q[b] as [128, HB] column tile (q_t[p, jb] = q[jb*128+p]), f32r
        qs = []
        for bi in range(BPC):
            v4 = wbuild.tile([HB, 128], f32r, tag="v4")
            nc.sync.dma_start(v4[:], hidden_d[bi, L - 1, :].rearrange("(f p) -> f p", p=128))
            tps = psgram.tile([128, HB], f32r, tag="gram")
            nc.tensor.matmul(tps[:], v4[:], ident_r[:HB, :HB], is_transpose=True)
            q_t = consts.tile([128, HB], f32r, tag=f"q{bi}", name=f"q{bi}")
            nc.scalar.copy(q_t[:], tps[:])
            qs.append(q_t)

        # ---- state: M^T per (b, jb), f32r, read directly by the PE ----
        mts = [[mtpool.tile([128, H], f32r, tag=f"mt{bi}_{jb}", name=f"mt{bi}_{jb}")
                for jb in range(HB)] for bi in range(BPC)]

        # ---- software-pipelined main loop over superchunks ----
        G = {}  # per-superchunk live tiles

        def prep(s):
            t0 = s * SC
            nrows1 = min(SC, T - t0) - 128  # rows in block 1 (127 on the last)
            st = {"k0": [], "k1": [], "kt": [], "nr": []}
            for bi in range(BPC):
                k0 = kpool.tile([128, H], f32r, tag=f"k0_{bi}", name=f"k0_{s}_{bi}")
                nc.sync.dma_start(k0[:], hidden_d[bi, t0:t0 + 128, :])
                k1 = kpool.tile([128, H], f32r, tag=f"k1_{bi}", name=f"k1_{s}_{bi}")
                # On the last superchunk row 127 of k1 is hidden[L-1] (the
                # query, not a scan key): DMA all 128 rows, then zero that row.
                nc.sync.dma_start(k1[:], hidden_d[bi, t0 + 128:t0 + SC, :])
                if nrows1 < 128:
                    nc.sync.dma_start(k1[nrows1:128, :], zrow_d[:, :])
                st["k0"].append(k0)
                st["k1"].append(k1)

                # ktb layout: [128, HB*256]; block hb at columns hb*256
                # (cols hb*256..+128 = K0^T block, +128..+256 = K1^T block)
                ktb = ktpool.tile([128, HB * 256], f32r, tag=f"kt{bi}", name=f"kt{s}_{bi}")
                for half in range(2):
                    ktps = pshi.tile([128, H], f32r, tag="big")
                    for i, (kk, hb) in enumerate(
                            [(k0, 2 * half), (k1, 2 * half), (k0, 2 * half + 1), (k1, 2 * half + 1)]):
                        nc.tensor.transpose(ktps[:, i * 128:(i + 1) * 128],
                                            kk[:, hb * 128:(hb + 1) * 128], ident_r[:])
                    nc.scalar.copy(ktb[:, half * 512:(half + 1) * 512], ktps[:])
                st["kt"].append(ktb)
            G[s] = st

        def aform(s):
            # Per-bi: pair-gram matmuls (PE) immediately followed by their
            # elementwise consumers, so the 2-deep gram PSUM ring never blocks.
            # g01[:, 0:256] = K0 [K0|K1]^T, [:, 256:512] = K1 [K0|K1]^T
            st = G[s]
            a0_all = chain.tile([128, H], bf16, tag="ak0")
            a1_all = chain.tile([128, H], bf16, tag="ak1")
            at_ps0 = pshi.tile([128, H], bf16, tag="big")
            at_ps1 = pshi.tile([128, H], bf16, tag="big")
            st["pt"], st["nr"] = [], []
            for bi in range(BPC):
                ktb = st["kt"][bi]
                g01 = psgram.tile([128, 512], f32, tag="gram")
                for hb in range(HB):
                    c = hb * 256
                    nc.tensor.matmul(g01[:, 0:256], ktb[:, c:c + 128], ktb[:, c:c + 256],
                                     start=(hb == 0), stop=(hb == HB - 1))
                for hb in range(HB):
                    c = hb * 256
                    nc.tensor.matmul(g01[:, 256:512], ktb[:, c + 128:c + 256], ktb[:, c:c + 256],
                                     start=(hb == 0), stop=(hb == HB - 1))
                # d_i = diag of G_ii via identity-masked row-accumulate (DVE)
                dd = small.tile([128, 2], f32, tag=f"dd{bi}")
                scr = small.tile([128, 128], f32, tag="dscr")
                nc.vector.scalar_tensor_tensor(scr[:], g01[:, 0:128], 1.0, ident_b[:],
                                               MULT, MULT, accum_out=dd[:, 0:1])
                scr2 = small.tile([128, 128], f32, tag="dscr")
                nc.vector.scalar_tensor_tensor(scr2[:], g01[:, 384:512], 1.0, ident_b[:],
                                               MULT, MULT, accum_out=dd[:, 1:2])
                rr = small.tile([128, 2], f32, tag=f"rr{bi}")
                nc.vector.tensor_scalar_add(dd[:], dd[:], EPS)
                nc.vector.reciprocal(rr[:], dd[:])
                nr = small.tile([128, 2], f32, tag=f"nr{bi}")
                nc.vector.tensor_scalar_mul(nr[:], rr[:], -1.0)
                st["nr"].append(nr)
                sl = slice(bi * 128, (bi + 1) * 128)
                nc.vector.scalar_tensor_tensor(a0_all[:, sl], g01[:, 0:128], rr[:, 0:1],
                                               smask[:], MULT, MULT)
                nc.vector.scalar_tensor_tensor(a1_all[:, sl], g01[:, 384:512], rr[:, 1:2],
                                               smask[:], MULT, MULT)
                pt = ptpool.tile([128, 128], f32r, tag=f"pt{bi}", name=f"pt{s}_{bi}")
                nc.scalar.copy(pt[:], g01[:, 128:256])
                st["pt"].append(pt)
                # per-bi at transposes: each waits only its own a_all slice
                nc.tensor.transpose(at_ps0[:, sl], a0_all[:, sl], ident_b[:])
                nc.tensor.transpose(at_ps1[:, sl], a1_all[:, sl], ident_b[:])
            # chain inits for both blocks
            for idx, at_ps in enumerate((at_ps0, at_ps1)):
                at_all = chain.tile([128, H], bf16, tag=f"atk{idx}")
                nc.scalar.copy(at_all[:], at_ps[:])
                gt = chain.tile([128, H], bf16, tag=f"g{idx}")
                nc.vector.tensor_sub(gt[:], identp[:], at_all[:])
                st[f"ak{idx}"], st[f"atk{idx}"], st[f"g{idx}"] = (
                    (a0_all, a1_all)[idx], at_all, gt)

        def chain_level(s, lev):
            st = G[s]
            for idx in range(2):
                ak, atk, gt = st[f"ak{idx}"], st[f"atk{idx}"], st[f"g{idx}"]
                sq1 = pshi.tile([128, H], f32, tag="big")
                for bi in range(BPC):
                    sl = slice(bi * 128, (bi + 1) * 128)
                    nc.tensor.matmul(sq1[:, sl], atk[:, sl], ak[:, sl], start=True, stop=True)
                ak2 = chain.tile([128, H], bf16, tag=f"ak{idx}")
                # DVE copy: runs parallel to ACT's atk2 copy, halving the
                # sq->copy->gps round-trip on the chain critical path
                nc.vector.tensor_copy(ak2[:], sq1[:])
                if lev < NLEV:
                    sq2 = pshi.tile([128, H], f32, tag="big")
                    for bi in range(BPC):
                        sl = slice(bi * 128, (bi + 1) * 128)
                        nc.tensor.matmul(sq2[:, sl], ak[:, sl], atk[:, sl], start=True, stop=True)
                    atk2 = chain.tile([128, H], bf16, tag=f"atk{idx}")
                    nc.scalar.copy(atk2[:], sq2[:])
                else:
                    atk2 = None
                gps = pshi.tile([128, H], f32, tag="big")
                for bi in range(BPC):
                    sl = slice(bi * 128, (bi + 1) * 128)
                    nc.tensor.matmul(gps[:, sl], ak2[:, sl], gt[:, sl], start=True, stop=True)
                if lev == NLEV:
                    g_nxt = gfpool.tile([128, H], f32r, tag=f"gf{idx}", name=f"gf{idx}_{s}")
                else:
                    g_nxt = chain.tile([128, H], bf16, tag=f"g{idx}")
                nc.vector.tensor_add(g_nxt[:], gps[:], gt[:])
                st[f"ak{idx}"], st[f"atk{idx}"], st[f"g{idx}"] = ak2, atk2, g_nxt

        def state0_ups(s):
            st = G[s]
            st["u0"], st["dl0"] = [], []
            for bi in range(BPC):
                ktb = st["kt"][bi]
                if s == 0:
                    u0 = st["k0"][bi]
                else:
                    ups = pshi.tile([128, H], f32, tag="big")
                    for hb in range(HB):
                        nc.tensor.matmul(ups[:], ktb[:, hb * 256:hb * 256 + 128],
                                         mts[bi][hb][:], start=(hb == 0), stop=(hb == HB - 1))
                    u0 = upool.tile([128, H], f32r, tag=f"u0_{bi}")
                    nc.vector.scalar_tensor_tensor(u0[:], ups[:], st["nr"][bi][:, 0:1],
                                                   st["k0"][bi][:], MULT, ADD)
                st["u0"].append(u0)

        def state0_dl(s):
            st = G[s]
            for bi in range(BPC):
                dps = pshi.tile([128, H], f32, tag="big")
                nc.tensor.matmul(dps[:], st["g0"][:, bi * 128:(bi + 1) * 128],
                                 st["u0"][bi][:], start=True, stop=True)
                dl0 = upool.tile([128, H], f32r, tag=f"dl0_{bi}")
                nc.scalar.copy(dl0[:], dps[:])
                st["dl0"].append(dl0)

        def state1_ups(s):
            st = G[s]
            st["u1"], st["dl1"] = [], []
            for bi in range(BPC):
                ktb = st["kt"][bi]
                ups = pshi.tile([128, H], f32, tag="big")
                if s > 0:
                    for hb in range(HB):
                        nc.tensor.matmul(ups[:], ktb[:, hb * 256 + 128:hb * 256 + 256],
                                         mts[bi][hb][:], start=(hb == 0), stop=False)
                nc.tensor.matmul(ups[:], st["pt"][bi][:], st["dl0"][bi][:],
                                 start=(s == 0), stop=True)
                u1 = upool.tile([128, H], f32r, tag=f"u1_{bi}")
                nc.vector.scalar_tensor_tensor(u1[:], ups[:], st["nr"][bi][:, 1:2],
                                               st["k1"][bi][:], MULT, ADD)
                st["u1"].append(u1)

        def state1_dl(s):
            st = G[s]
            for bi in range(BPC):
                dps = pshi.tile([128, H], f32, tag="big")
                nc.tensor.matmul(dps[:], st["g1"][:, bi * 128:(bi + 1) * 128],
                                 st["u1"][bi][:], start=True, stop=True)
                dl1 = upool.tile([128, H], f32r, tag=f"dl1_{bi}")
                nc.scalar.copy(dl1[:], dps[:])
                st["dl1"].append(dl1)

        def mupd(s, bis):
            st = G[s]
            for bi in bis:
                for jb in range(HB):
                    mps = pshi.tile([128, H], f32, tag="big")
                    nc.tensor.matmul(mps[:], st["k0"][bi][:, jb * 128:(jb + 1) * 128],
                                     st["dl0"][bi][:], start=True, stop=False)
                    nc.tensor.matmul(mps[:], st["k1"][bi][:, jb * 128:(jb + 1) * 128],
                                     st["dl1"][bi][:], start=False, stop=True)
                    if s == 0:
                        nc.vector.tensor_copy(mts[bi][jb][:], mps[:])
                    else:
                        nc.vector.tensor_add(mts[bi][jb][:], mps[:], mts[bi][jb][:])

        prep(0)
        aform(0)
        for lev in range(1, NLEV + 1):
            chain_level(0, lev)
        for s in range(NSC):
            nxt = s + 1 if s + 1 < NSC else None
            if nxt is not None:
                prep(nxt)
            # u-stts (DVE) are emitted ahead of the next superchunk's
            # elementwise so the dl matmuls don't stall behind them.
            state0_ups(s)
            if nxt is not None:
                aform(nxt)
            state0_dl(s)
            state1_ups(s)
            state1_dl(s)
            # both chain levels sit between the dl1 matmuls and mupd: the dl1
            # ACT copies lead the ACT queue while the PE chews bf16 chain work.
            if nxt is not None:
                chain_level(nxt, 1)
                chain_level(nxt, 2)
            mupd(s, [0, 1])
            if nxt is not None:
                chain_level(nxt, 3)
            mupd(s, [2, 3])
            prev = s - 1
            if prev in G:
                del G[prev]

        # ---- finale: ctx = M q (row form); out = ctx W^T + b ----
        for bi in range(BPC):
            cps = pshi.tile([1, H], f32, tag="big")
            for jb in range(HB):
                nc.tensor.matmul(cps[:], qs[bi][:, jb:jb + 1], mts[bi][jb][:],
                                 start=(jb == 0), stop=(jb == HB - 1))
            ctx_row = small.tile([1, H], f32, tag="ctxrow")
            nc.scalar.copy(ctx_row[:], cps[:])
            ctxT = small.tile([128, HB], f32, tag="ctxT")
            for ib in range(HB):
                tp2 = psgram.tile([128, 1], f32, tag="gram")
                nc.tensor.transpose(tp2[:], ctx_row[:, ib * 128:(ib + 1) * 128], ident_f[:1, :1])
                nc.scalar.copy(ctxT[:, ib:ib + 1], tp2[:])
            ops_ = pshi.tile([1, H], f32, tag="big")
            for ib in range(HB):
                nc.tensor.matmul(ops_[:], ctxT[:, ib:ib + 1], wt[ib][:],
                                 start=(ib == 0), stop=(ib == HB - 1))
            out_row = small.tile([1, H], f32, tag="outrow")
            nc.vector.tensor_add(out_row[:], ops_[:], bias_row[:])
            nc.sync.dma_start(out_d[bi, :][None, :], out_row[:])

    _legalize_waits(nc)
    return nc


def _legalize_waits(nc, max_waits=1):
    """This toolchain's walrus encodes at most one semaphore wait per
    instruction. Hoist extra waits onto standalone EventSemaphore
    instructions on the same engine queue, immediately before the owner."""
    import json as _json
    m = _json.loads(bytes(nc.to_json_bytes()))
    n_fix = 0
    for fn in m["functions"]:
        for blk in fn["blocks"]:
            out = []
            for ins in blk.get("instructions", []):
                si = ins.get("sync_info") or {}
                waits = si.get("on_wait") or []
                if len(waits) > max_waits and ins.get("opcode") != "EventSemaphore":
                    extra, keep = waits[:-max_waits], waits[-max_waits:]
                    for i, w in enumerate(extra):
                        out.append({
                            "name": f"{ins['name']}-w{i}",
                            "engine": ins["engine"],
                            "opcode": "EventSemaphore",
                            "ins": [], "outs": [],
                            "sync_info": {"on_wait": [w], "on_update": []},
                        })
                    si["on_wait"] = keep
                    ins["sync_info"] = si
                    n_fix += 1
                out.append(ins)
            blk["instructions"] = out
    nc.m = mybir.module_from_json_bytes(_json.dumps(m).encode())
    return n_fix


def kernel(hidden: np.ndarray, W: np.ndarray, b: np.ndarray) -> np.ndarray:
    if "nc" not in _cached:
        _cached["nc"] = _build_program()
    nc = _cached["nc"]

    hidden = np.ascontiguousarray(hidden, dtype=np.float32)
    W = np.ascontiguousarray(W, dtype=np.float32)
    b = np.ascontiguousarray(b, dtype=np.float32)

    in_maps = []
    for ci in range(NCORES):
        in_maps.append({
            "hidden": hidden[ci * BPC:(ci + 1) * BPC],
            "W": W,
            "bvec": b,
            "zrow": np.zeros((1, H), np.float32),
        })
    res = run_bass_kernel_spmd(nc, in_maps, core_ids=list(range(NCORES)))
    _cached["last_results"] = res
    out = np.concatenate([res.results[ci]["out"] for ci in range(NCORES)], axis=0)
    return out.astype(np.float32)


if __name__ == "__main__":
    rng = np.random.default_rng(0)
    h = rng.standard_normal((B, L, H), dtype=np.float32)
    w = rng.standard_normal((H, H), dtype=np.float32) * (1.0 / np.sqrt(H))
    bb = np.zeros((H,), np.float32)
    o = kernel(h, w, bb)
    print(o.shape, o.dtype)


# revision 6
# speedup vs baseline: 1.0046x; 1.0046x over previous
"""DeltaRule memory scan kernel for Trainium2, 8 NeuronCores, data-parallel over batch.

Reference semantics (per batch element, H=512, L=2048):
    M_0 = 0  [H,H]
    for t in 0..L-2:   k = hidden[t]
        d = k.k + eps; delta = k - (M k)/d; M += outer(delta, k)
    out = (M @ hidden[L-1]) @ W.T + b

Implementation: superchunks of 256 keys processed as 2x128 blocks (UT transform
per 128-block + cross-block correction). All state matmuls run in float32r
(1 cycle/row at free-dim >= 256); the intra-block inverse chain runs in bf16.

Per superchunk with K0,K1 [128,H], r_i = 1/(rowsum(Ki^2)+eps):
    G0 = K0 [K0|K1]^T, G1 = K1 [K0|K1]^T          (pair gram, f32r, ap256)
    d_i = diag block of G_i ; A_i = strict_tril(diag(r_i) G_ii) ; PT = G0[:,128:]
    T_i^T ~= Neumann chain of A_i^T (exact through A^31)
    U0 = T0 (K0 - diag(r0) K0 M^T)
    V1 = K1 M^T + PT^T U0 ;  U1 = T1 (K1 - diag(r1) V1)
    M^T += K0^T U0 + K1^T U1                       (PSUM-merged, one add per jb)
"""
import sys
import numpy as np
from contextlib import ExitStack

sys.path.insert(0, "/opt/trn_rl_repo")

import concourse.bass as bass
import concourse.mybir as mybir
from concourse import tile
from concourse.bass_utils import run_bass_kernel_spmd
from concourse.masks import make_identity, make_lower_triangular

B, L, H = 32, 2048, 512
NCORES = 8
BPC = B // NCORES          # 4 batch elements per core
SC = 256                   # superchunk (2 blocks of 128)
T = L - 1                  # 2047 scan steps
NSC = (T + SC - 1) // SC   # 8 (last superchunk has 255 valid rows)
NLEV = 3                   # Neumann levels -> exact through A^15 (err ~||A||^16)
EPS = 1e-6
HB = H // 128              # 4 h-blocks

f32 = mybir.dt.float32
f32r = mybir.dt.float32r
bf16 = mybir.dt.bfloat16
MULT = mybir.AluOpType.mult
ADD = mybir.AluOpType.add

_cached = {}


def _build_program():
    nc = bass.Bass(target_bir_lowering=False, debug=False)

    hidden_d = nc.declare_dram_parameter("hidden", [BPC, L, H], f32r, isOutput=False)
    w_d = nc.declare_dram_parameter("W", [H, H], f32, isOutput=False)
    b_d = nc.declare_dram_parameter("bvec", [H], f32, isOutput=False)
    zrow_d = nc.declare_dram_parameter("zrow", [1, H], f32r, isOutput=False)
    out_d = nc.declare_dram_parameter("out", [BPC, H], f32, isOutput=True)

    with tile.TileContext(nc) as tc, ExitStack() as ctx:
        consts = ctx.enter_context(tc.tile_pool(name="consts", bufs=1))
        wbuild = ctx.enter_context(tc.tile_pool(name="wbuild", bufs=2))
        mtpool = ctx.enter_context(tc.tile_pool(name="mt", bufs=1))
        kpool = ctx.enter_context(tc.tile_pool(name="k", bufs=2))
        ktpool = ctx.enter_context(tc.tile_pool(name="kt", bufs=2))
        chain = ctx.enter_context(tc.tile_pool(name="chain", bufs=3))
        gfpool = ctx.enter_context(tc.tile_pool(name="gf", bufs=2))
        upool = ctx.enter_context(tc.tile_pool(name="u", bufs=1))
        ptpool = ctx.enter_context(tc.tile_pool(name="pt", bufs=2))
        small = ctx.enter_context(tc.tile_pool(name="small", bufs=2))
        psgram = ctx.enter_context(tc.tile_pool(name="psgram", bufs=3, space="PSUM"))
        pshi = ctx.enter_context(tc.tile_pool(name="pshi", bufs=5, space="PSUM"))

        # ---- constants ----
        ident_f = consts.tile([128, 128], f32, tag="identf")
        make_identity(nc, ident_f[:])
        ident_b = consts.tile([128, 128], bf16, tag="identb")
        make_identity(nc, ident_b[:])
        ident_r = consts.tile([128, 128], f32r, tag="identr")
        nc.scalar.copy(ident_r[:], ident_f[:])
        # packed identity: I in each of the 4 b-slices
        identp = consts.tile([128, H], bf16, tag="identp")
        for bi in range(BPC):
            nc.vector.tensor_copy(identp[:, bi * 128:(bi + 1) * 128], ident_b[:])
        smask = consts.tile([128, 128], f32, tag="smask")
        make_lower_triangular(nc, smask[:], val=1.0, diag=False)

        # W^T: WT[ib][i', o] = W[o, ib*128+i']  (f32, used once in the finale)
        wt = [consts.tile([128, H], f32, tag=f"wt{ib}", name=f"wt{ib}") for ib in range(HB)]
        for op in range(HB):
            wsb = wbuild.tile([128, H], f32, tag="wsb")
            nc.sync.dma_start(wsb[:], w_d[op * 128:(op + 1) * 128, :])
            for ib in range(HB):
                tps = psgram.tile([128, 128], f32, tag="gram")
                nc.tensor.transpose(tps[:], wsb[:, ib * 128:(ib + 1) * 128], ident_f[:])
                nc.scalar.copy(wt[ib][:, op * 128:(op + 1) * 128], tps[:])

        bias_row = consts.tile([1, H], f32, tag="biasrow")
        nc.sync.dma_start(bias_row[:], b_d[None, :])

        # q[b] as [128, HB] column tile (q_t[p, jb] = q[jb*128+p]), f32r
        qs = []
        for bi in range(BPC):
            v4 = wbuild.tile([HB, 128], f32r, tag="v4")
            nc.sync.dma_start(v4[:], hidden_d[bi, L - 1, :].rearrange("(f p) -> f p", p=128))
            tps = psgram.tile([128, HB], f32r, tag="gram")
            nc.tensor.matmul(tps[:], v4[:], ident_r[:HB, :HB], is_transpose=True)
            q_t = consts.tile([128, HB], f32r, tag=f"q{bi}", name=f"q{bi}")
            nc.scalar.copy(q_t[:], tps[:])
            qs.append(q_t)

        # ---- state: M^T per (b, jb), f32r, read directly by the PE ----
        mts = [[mtpool.tile([128, H], f32r, tag=f"mt{bi}_{jb}", name=f"mt{bi}_{jb}")
                for jb in range(HB)] for bi in range(BPC)]

        # ---- software-pipelined main loop over superchunks ----
        G = {}  # per-superchunk live tiles

        def prep(s):
            t0 = s * SC
            nrows1 = min(SC, T - t0) - 128  # rows in block 1 (127 on the last)
            st = {"k0": [], "k1": [], "kt": [], "nr": []}
            for bi in range(BPC):
                k0 = kpool.tile([128, H], f32r, tag=f"k0_{bi}", name=f"k0_{s}_{bi}")
                nc.sync.dma_start(k0[:], hidden_d[bi, t0:t0 + 128, :])
                k1 = kpool.tile([128, H], f32r, tag=f"k1_{bi}", name=f"k1_{s}_{bi}")
                # On the last superchunk row 127 of k1 is hidden[L-1] (the
                # query, not a scan key): DMA all 128 rows, then zero that row.
                nc.sync.dma_start(k1[:], hidden_d[bi, t0 + 128:t0 + SC, :])
                if nrows1 < 128:
                    nc.sync.dma_start(k1[nrows1:128, :], zrow_d[:, :])
                st["k0"].append(k0)
                st["k1"].append(k1)

                # ktb layout: [128, HB*256]; block hb at columns hb*256
                # (cols hb*256..+128 = K0^T block, +128..+256 = K1^T block)
                ktb = ktpool.tile([128, HB * 256], f32r, tag=f"kt{bi}", name=f"kt{s}_{bi}")
                for half in range(2):
                    ktps = pshi.tile([128, H], f32r, tag="big")
                    for i, (kk, hb) in enumerate(
                            [(k0, 2 * half), (k1, 2 * half), (k0, 2 * half + 1), (k1, 2 * half + 1)]):
                        nc.tensor.transpose(ktps[:, i * 128:(i + 1) * 128],
                                            kk[:, hb * 128:(hb + 1) * 128], ident_r[:])
                    nc.scalar.copy(ktb[:, half * 512:(half + 1) * 512], ktps[:])
                st["kt"].append(ktb)
            G[s] = st

        def aform(s):
            # Per-bi: pair-gram matmuls (PE) immediately followed by their
            # elementwise consumers, so the 2-deep gram PSUM ring never blocks.
            # g01[:, 0:256] = K0 [K0|K1]^T, [:, 256:512] = K1 [K0|K1]^T
            st = G[s]
            a0_all = chain.tile([128, H], bf16, tag="ak0")
            a1_all = chain.tile([128, H], bf16, tag="ak1")
            at_ps0 = pshi.tile([128, H], bf16, tag="big")
            at_ps1 = pshi.tile([128, H], bf16, tag="big")
            st["pt"], st["nr"] = [], []
            for bi in range(BPC):
                ktb = st["kt"][bi]
                g01 = psgram.tile([128, 512], f32, tag="gram")
                for hb in range(HB):
                    c = hb * 256
                    nc.tensor.matmul(g01[:, 0:256], ktb[:, c:c + 128], ktb[:, c:c + 256],
                                     start=(hb == 0), stop=(hb == HB - 1))
                for hb in range(HB):
                    c = hb * 256
                    nc.tensor.matmul(g01[:, 256:512], ktb[:, c + 128:c + 256], ktb[:, c:c + 256],
                                     start=(hb == 0), stop=(hb == HB - 1))
                # d_i = diag of G_ii via identity-masked row-accumulate (DVE)
                dd = small.tile([128, 2], f32, tag=f"dd{bi}")
                scr = small.tile([128, 128], f32, tag="dscr")
                nc.vector.scalar_tensor_tensor(scr[:], g01[:, 0:128], 1.0, ident_b[:],
                                               MULT, MULT, accum_out=dd[:, 0:1])
                scr2 = small.tile([128, 128], f32, tag="dscr")
                nc.vector.scalar_tensor_tensor(scr2[:], g01[:, 384:512], 1.0, ident_b[:],
                                               MULT, MULT, accum_out=dd[:, 1:2])
                rr = small.tile([128, 2], f32, tag=f"rr{bi}")
                nc.vector.tensor_scalar_add(dd[:], dd[:], EPS)
                nc.vector.reciprocal(rr[:], dd[:])
                nr = small.tile([128, 2], f32, tag=f"nr{bi}")
                nc.vector.tensor_scalar_mul(nr[:], rr[:], -1.0)
                st["nr"].append(nr)
                sl = slice(bi * 128, (bi + 1) * 128)
                nc.vector.scalar_tensor_tensor(a0_all[:, sl], g01[:, 0:128], rr[:, 0:1],
                                               smask[:], MULT, MULT)
                nc.vector.scalar_tensor_tensor(a1_all[:, sl], g01[:, 384:512], rr[:, 1:2],
                                               smask[:], MULT, MULT)
                pt = ptpool.tile([128, 128], f32r, tag=f"pt{bi}", name=f"pt{s}_{bi}")
                nc.scalar.copy(pt[:], g01[:, 128:256])
                st["pt"].append(pt)
                # per-bi at transposes: each waits only its own a_all slice
                nc.tensor.transpose(at_ps0[:, sl], a0_all[:, sl], ident_b[:])
                nc.tensor.transpose(at_ps1[:, sl], a1_all[:, sl], ident_b[:])
            # chain inits for both blocks
            for idx, at_ps in enumerate((at_ps0, at_ps1)):
                at_all = chain.tile([128, H], bf16, tag=f"atk{idx}")
                nc.scalar.copy(at_all[:], at_ps[:])
                gt = chain.tile([128, H], bf16, tag=f"g{idx}")
                nc.vector.tensor_sub(gt[:], identp[:], at_all[:])
                st[f"ak{idx}"], st[f"atk{idx}"], st[f"g{idx}"] = (
                    (a0_all, a1_all)[idx], at_all, gt)

        def chain_level(s, lev):
            # phase-major across the two chains: all sq1 matmuls, then all sq2,
            # then all gps — the PE covers each ak2/atk2 ACT copy with the
            # other chain's matmuls instead of stalling on its own.
            st = G[s]
            sq1s, ak2s, atk2s, gpss = [], [], [], []
            for idx in range(2):
                ak, atk = st[f"ak{idx}"], st[f"atk{idx}"]
                sq1 = pshi.tile([128, H], f32, tag="big")
                for bi in range(BPC):
                    sl = slice(bi * 128, (bi + 1) * 128)
                    nc.tensor.matmul(sq1[:, sl], atk[:, sl], ak[:, sl], start=True, stop=True)
                sq1s.append(sq1)
                ak2 = chain.tile([128, H], bf16, tag=f"ak{idx}")
                nc.scalar.copy(ak2[:], sq1[:])
                ak2s.append(ak2)
            if lev < NLEV:
                for idx in range(2):
                    ak, atk = st[f"ak{idx}"], st[f"atk{idx}"]
                    sq2 = pshi.tile([128, H], f32, tag="big")
                    for bi in range(BPC):
                        sl = slice(bi * 128, (bi + 1) * 128)
                        nc.tensor.matmul(sq2[:, sl], ak[:, sl], atk[:, sl], start=True, stop=True)
                    atk2 = chain.tile([128, H], bf16, tag=f"atk{idx}")
                    nc.scalar.copy(atk2[:], sq2[:])
                    atk2s.append(atk2)
            else:
                atk2s = [None, None]
            for idx in range(2):
                gt = st[f"g{idx}"]
                gps = pshi.tile([128, H], f32, tag="big")
                for bi in range(BPC):
                    sl = slice(bi * 128, (bi + 1) * 128)
                    nc.tensor.matmul(gps[:, sl], ak2s[idx][:, sl], gt[:, sl], start=True, stop=True)
                gpss.append(gps)
            for idx in range(2):
                gt = st[f"g{idx}"]
                if lev == NLEV:
                    g_nxt = gfpool.tile([128, H], f32r, tag=f"gf{idx}", name=f"gf{idx}_{s}")
                else:
                    g_nxt = chain.tile([128, H], bf16, tag=f"g{idx}")
                nc.vector.tensor_add(g_nxt[:], gpss[idx][:], gt[:])
                st[f"ak{idx}"], st[f"atk{idx}"], st[f"g{idx}"] = ak2s[idx], atk2s[idx], g_nxt

        def state0_ups(s):
            st = G[s]
            st["u0"], st["dl0"] = [], []
            for bi in range(BPC):
                ktb = st["kt"][bi]
                if s == 0:
                    u0 = st["k0"][bi]
                else:
                    ups = pshi.tile([128, H], f32, tag="big")
                    for hb in range(HB):
                        nc.tensor.matmul(ups[:], ktb[:, hb * 256:hb * 256 + 128],
                                         mts[bi][hb][:], start=(hb == 0), stop=(hb == HB - 1))
                    u0 = upool.tile([128, H], f32r, tag=f"u0_{bi}")
                    nc.vector.scalar_tensor_tensor(u0[:], ups[:], st["nr"][bi][:, 0:1],
                                                   st["k0"][bi][:], MULT, ADD)
                st["u0"].append(u0)

        def state0_dl(s):
            st = G[s]
            for bi in range(BPC):
                dps = pshi.tile([128, H], f32, tag="big")
                nc.tensor.matmul(dps[:], st["g0"][:, bi * 128:(bi + 1) * 128],
                                 st["u0"][bi][:], start=True, stop=True)
                dl0 = upool.tile([128, H], f32r, tag=f"dl0_{bi}")
                nc.scalar.copy(dl0[:], dps[:])
                st["dl0"].append(dl0)

        def state1_ups(s):
            st = G[s]
            st["u1"], st["dl1"] = [], []
            for bi in range(BPC):
                ktb = st["kt"][bi]
                ups = pshi.tile([128, H], f32, tag="big")
                if s > 0:
                    for hb in range(HB):
                        nc.tensor.matmul(ups[:], ktb[:, hb * 256 + 128:hb * 256 + 256],
                                         mts[bi][hb][:], start=(hb == 0), stop=False)
                nc.tensor.matmul(ups[:], st["pt"][bi][:], st["dl0"][bi][:],
                                 start=(s == 0), stop=True)
                u1 = upool.tile([128, H], f32r, tag=f"u1_{bi}")
                nc.vector.scalar_tensor_tensor(u1[:], ups[:], st["nr"][bi][:, 1:2],
                                               st["k1"][bi][:], MULT, ADD)
                st["u1"].append(u1)

        def state1_dl(s):
            st = G[s]
            for bi in range(BPC):
                dps = pshi.tile([128, H], f32, tag="big")
                nc.tensor.matmul(dps[:], st["g1"][:, bi * 128:(bi + 1) * 128],
                                 st["u1"][bi][:], start=True, stop=True)
                dl1 = upool.tile([128, H], f32r, tag=f"dl1_{bi}")
                nc.scalar.copy(dl1[:], dps[:])
                st["dl1"].append(dl1)

        def mupd(s, bis):
            st = G[s]
            for bi in bis:
                for jb in range(HB):
                    mps = pshi.tile([128, H], f32, tag="big")
                    nc.tensor.matmul(mps[:], st["k0"][bi][:, jb * 128:(jb + 1) * 128],
                                     st["dl0"][bi][:], start=True, stop=False)
                    nc.tensor.matmul(mps[:], st["k1"][bi][:, jb * 128:(jb + 1) * 128],
                                     st["dl1"][bi][:], start=False, stop=True)
                    if s == 0:
                        nc.vector.tensor_copy(mts[bi][jb][:], mps[:])
                    else:
                        nc.vector.tensor_add(mts[bi][jb][:], mps[:], mts[bi][jb][:])

        prep(0)
        aform(0)
        for lev in range(1, NLEV + 1):
            chain_level(0, lev)
        for s in range(NSC):
            nxt = s + 1 if s + 1 < NSC else None
            if nxt is not None:
                prep(nxt)
            # u-stts (DVE) are emitted ahead of the next superchunk's
            # elementwise so the dl matmuls don't stall behind them.
            state0_ups(s)
            if nxt is not None:
                aform(nxt)
            state0_dl(s)
            state1_ups(s)
            state1_dl(s)
            # both chain levels sit between the dl1 matmuls and mupd: the dl1
            # ACT copies lead the ACT queue while the PE chews bf16 chain work.
            if nxt is not None:
                chain_level(nxt, 1)
                chain_level(nxt, 2)
            mupd(s, [0, 1])
            if nxt is not None:
                chain_level(nxt, 3)
            mupd(s, [2, 3])
            prev = s - 1
            if prev in G:
                del G[prev]

        # ---- finale: ctx = M q (row form); out = ctx W^T + b ----
        for bi in range(BPC):
            cps = pshi.tile([1, H], f32, tag="big")
            for jb in range(HB):
                nc.tensor.matmul(cps[:], qs[bi][:, jb:jb + 1], mts[bi][jb][:],
                                 start=(jb == 0), stop=(jb == HB - 1))
            ctx_row = small.tile([1, H], f32, tag="ctxrow")
            nc.scalar.copy(ctx_row[:], cps[:])
            ctxT = small.tile([128, HB], f32, tag="ctxT")
            for ib in range(HB):
                tp2 = psgram.tile([128, 1], f32, tag="gram")
                nc.tensor.transpose(tp2[:], ctx_row[:, ib * 128:(ib + 1) * 128], ident_f[:1, :1])
                nc.scalar.copy(ctxT[:, ib:ib + 1], tp2[:])
            ops_ = pshi.tile([1, H], f32, tag="big")
            for ib in range(HB):
                nc.tensor.matmul(ops_[:], ctxT[:, ib:ib + 1], wt[ib][:],
                                 start=(ib == 0), stop=(ib == HB - 1))
            out_row = small.tile([1, H], f32, tag="outrow")
            nc.vector.tensor_add(out_row[:], ops_[:], bias_row[:])
            nc.sync.dma_start(out_d[bi, :][None, :], out_row[:])

    _legalize_waits(nc)
    return nc


def _legalize_waits(nc, max_waits=1):
    """This toolchain's walrus encodes at most one semaphore wait per
    instruction. Hoist extra waits onto standalone EventSemaphore
    instructions on the same engine queue, immediately before the owner."""
    import json as _json
    m = _json.loads(bytes(nc.to_json_bytes()))
    n_fix = 0
    for fn in m["functions"]:
        for blk in fn["blocks"]:
            out = []
            for ins in blk.get("instructions", []):
                si = ins.get("sync_info") or {}
                waits = si.get("on_wait") or []
                if len(waits) > max_waits and ins.get("opcode") != "EventSemaphore":
                    extra, keep = waits[:-max_waits], waits[-max_waits:]
                    for i, w in enumerate(extra):
                        out.append({
                            "name": f"{ins['name']}-w{i}",
                            "engine": ins["engine"],
                            "opcode": "EventSemaphore",
                            "ins": [], "outs": [],
                            "sync_info": {"on_wait": [w], "on_update": []},
                        })
                    si["on_wait"] = keep
                    ins["sync_info"] = si
                    n_fix += 1
                out.append(ins)
            blk["instructions"] = out
    nc.m = mybir.module_from_json_bytes(_json.dumps(m).encode())
    return n_fix


def kernel(hidden: np.ndarray, W: np.ndarray, b: np.ndarray) -> np.ndarray:
    if "nc" not in _cached:
        _cached["nc"] = _build_program()
    nc = _cached["nc"]

    hidden = np.ascontiguousarray(hidden, dtype=np.float32)
    W = np.ascontiguousarray(W, dtype=np.float32)
    b = np.ascontiguousarray(b, dtype=np.float32)

    in_maps = []
    for ci in range(NCORES):
        in_maps.append({
            "hidden": hidden[ci * BPC:(ci + 1) * BPC],
            "W": W,
            "bvec": b,
            "zrow": np.zeros((1, H), np.float32),
        })
    res = run_bass_kernel_spmd(nc, in_maps, core_ids=list(range(NCORES)))
    _cached["last_results"] = res
    out = np.concatenate([res.results[ci]["out"] for ci in range(NCORES)], axis=0)
    return out.astype(np.float32)


if __name__ == "__main__":
    rng = np.random.default_rng(0)
    h = rng.standard_normal((B, L, H), dtype=np.float32)
    w = rng.standard_normal((H, H), dtype=np.float32) * (1.0 / np.sqrt(H))
    bb = np.zeros((H,), np.float32)
    o = kernel(h, w, bb)
    print(o.shape, o.dtype)
